# revision 56
# baseline (speedup 1.0000x reference)
"""Trainium2 Bass kernel for nn_LocalInferenceModeling (cross-attention enhance).

Reference computation (per batch b):
    e = x1 @ x2^T                                  [L, L]
    a12 = softmax_j(e + m2[j]);  x1t = a12 @ x2    [L, H]
    a21 = softmax_i(e^T + m1[i]); x2t = a21 @ x1   [L, H]
    y1 = concat([x1, x1t, x1 - x1t, x1 * x1t], -1) [L, 4H]
    y2 = concat([x2, x2t, x2 - x2t, x2 * x2t], -1)

Sharding: batch dim B=32 split across 8 NeuronCores (4 batches/core),
no communication.  Masks (0 / -1e30 rows from seq_lengths) are computed
host-side and passed as extra inputs.

Per-core dataflow (per batch):
  - one DMA per input tensor loads the whole batch into a [128, 4096]
    tile (partition p holds rows {128a+p} as column blocks a)
  - PE-transpose -> x1T, x2T [8x(128,512)] (h on partitions) in f32, with
    f32r rounding copies to SBUF (walrus requires f32r operands to be
    produced rounded, not bitcast)
  - e12 [i,j]: matmul(lhsT=x1T, rhs=x2T) accum over 8 h-tiles (f32r,
    1 cyc/row); clean copy to SBUF, then a bf16 rank-1 mask row
    (ones^T @ m2) into the same PSUM bank and a masked copy for stats
  - e21 [j,i]: rank-1 m1 into a fresh PSUM bank, then accumulate PE
    transposes of the clean e12 SBUF copy (exact, 5x cheaper than a
    second full matmul)
  - masked softmax per orientation: reduce_max(negate) -> Exp(bias=-max,
    accum_out=z) -> reciprocal; probs kept UNNORMALIZED in bf16 (1/z is
    applied after stage 2; bf16 noise on O(1) probabilities is harmless)
  - PE-transpose probs (bf16, 1 cyc/row) -> p12T/p21T
  - stage 2: tilde = probsT^T @ xv (bf16 x bf16, where xv is a bf16 copy
    of x), normalized by 1/z during the PSUM->SBUF copy straight into a
    bf16 [128, 2x4H] output tile along with x_bar / sub / mul slices
  - one bf16 DMA per 256 output rows (halves output HBM traffic vs fp32;
    host converts back to fp32)

The batch loop is software-pipelined at orientation granularity so the
in-order PE stream never stalls on the DVE/Act softmax-stats chain and
output DMAs are spread across each iteration to keep the DMA engines
(the binding resource: ~48 MB/core at 360 GB/s) continuously fed:
  iter k: s5b(k-1); loads(k+1); {out2(k-1) x xT-transposes(k)};
          {held out1(k-1) x e12(k)}; e21(k); s5a(k); out1(k) [1 held]
"""

import sys

import numpy as np

sys.path.insert(0, "/opt/trn_rl_repo")

from contextlib import ExitStack

import concourse.bass as bass
import concourse.bacc as bacc
import concourse.mybir as mybir
from concourse import masks
from concourse.bass_utils import run_bass_kernel_spmd
from concourse.tile import TileContext

B, L, H = 32, 512, 1024
NCORES = 8
BPC = B // NCORES  # batches per core
NEG = np.float32(-1.0e30)

F32 = mybir.dt.float32
F32R = mybir.dt.float32r
BF16 = mybir.dt.bfloat16
F16 = mybir.dt.float16

NT = L // 128  # 4 partition tiles per L
HT = H // 128  # 8 partition tiles per H
Exp = mybir.ActivationFunctionType.Exp
Copy = mybir.ActivationFunctionType.Copy
AX = mybir.AxisListType.X

_NC_CACHE = {}


def build_nc():
    nc = bacc.Bacc(None, target_bir_lowering=False)
    x1 = nc.dram_tensor("x1", [BPC, L, H], F16, kind="ExternalInput")
    x2 = nc.dram_tensor("x2", [BPC, L, H], F16, kind="ExternalInput")
    m1 = nc.dram_tensor("m1", [BPC, L], F16, kind="ExternalInput")
    m2 = nc.dram_tensor("m2", [BPC, L], F16, kind="ExternalInput")
    y1 = nc.dram_tensor("y1", [BPC, L, 4 * H], F16, kind="ExternalOutput")
    y2 = nc.dram_tensor("y2", [BPC, L, 4 * H], F16, kind="ExternalOutput")

    with TileContext(nc) as tc, ExitStack() as ctx:
        from concourse.tile import add_dep_helper

        const = ctx.enter_context(tc.tile_pool(name="const", bufs=1))
        ident = const.tile([128, 128], F32)
        masks.make_identity(nc, ident[:])
        ones = const.tile([1, 128], F32)
        nc.vector.memset(ones[:], 1.0)
        # bf16 identity / ones for the bf16-rate matmuls (1 cyc/row); walrus
        # rejects f32-produced data bitcast as f32r and mixed f32r/bf16
        # operands, so low-precision operands get real bf16 copies instead
        identb = const.tile([128, 128], F16, name="identb")
        nc.gpsimd.tensor_copy(identb[:], ident[:])
        onesb = const.tile([1, 128], F16, name="onesb")
        nc.vector.memset(onesb[:], 1.0)

        xp = ctx.enter_context(tc.tile_pool(name="xp", bufs=3))
        xtp = ctx.enter_context(tc.tile_pool(name="xtp", bufs=HT))
        ecp = ctx.enter_context(tc.tile_pool(name="ecp", bufs=NT + 1))
        emp = ctx.enter_context(tc.tile_pool(name="emp", bufs=3))
        pp = ctx.enter_context(tc.tile_pool(name="pp", bufs=NT + 1))
        ptp = ctx.enter_context(tc.tile_pool(name="ptp", bufs=NT + 1))
        st = ctx.enter_context(tc.tile_pool(name="st", bufs=8 * NT))
        yp = ctx.enter_context(tc.tile_pool(name="yp", bufs=3))
        mrp = ctx.enter_context(tc.tile_pool(name="mrp", bufs=1))
        psA = ctx.enter_context(tc.tile_pool(name="psA", bufs=3, space="PSUM"))
        psTP = ctx.enter_context(tc.tile_pool(name="psTP", bufs=2, space="PSUM"))
        psB = ctx.enter_context(tc.tile_pool(name="psB", bufs=2, space="PSUM"))
        psS = ctx.enter_context(tc.tile_pool(name="psS", bufs=1, space="PSUM"))
        scratch = psS.tile([32, 32], F32, name="scratch", tag="scratch")

        gates = {"psA": [], "psTP": [], "psB": []}

        def touch(ap):
            # Tiny PE transpose reading `ap` so the PE engine observes the
            # producer's sem tick; real matmuls then carry at most one sync
            # wait (walrus can encode only one on self-loading matmuls).
            a32 = ap[0:32, 0:32]
            if a32.dtype != F32:
                if mybir.dt.size(a32.dtype) == 2:
                    a32 = ap[0:32, 0:64].bitcast(F32)
                else:
                    a32 = a32.bitcast(F32)
            with tc.high_priority(offset=200):
                return nc.tensor.transpose(scratch[:], a32, ident[0:32, 0:32])

        def gate(tag, bufs, first_inst):
            # Order the group's first PE write after the touch that observed
            # the release of the slot it reuses (bufs groups back).
            hist = gates[tag]
            k = len(hist)
            if k >= bufs and hist[k - bufs] is not None:
                add_dep_helper(first_inst.ins, hist[k - bufs].ins, sync=False,
                               reason="psum slot gate")
            hist.append(None)  # placeholder until release touch known
            return k

        def set_gate(tag, k, tinst):
            gates[tag][k] = tinst

        touch(ident)
        nc.tensor.matmul(scratch[0:32, 0:1], ones[0:1, 0:32], ones[0:1, 0:1],
                         start=True, stop=True)

        # bf16 mask rows for the rank-1 mask matmuls (bf16 runs 1 cyc/row
        # and -1e30 only needs its magnitude)
        m1b = mrp.tile([1, BPC * L], F16, name="m1b", tag="m1b")
        m2b = mrp.tile([1, BPC * L], F16, name="m2b", tag="m2b")
        nc.sync.dma_start(m1b[:1, :], m1.rearrange("b l -> (b l)")[None, :])
        nc.sync.dma_start(m2b[:1, :], m2.rearrange("b l -> (b l)")[None, :])

        # [BPC, 128, NT*H] view: partition p of batch b holds rows {128a+p}
        # as column blocks a — one DMA loads a whole batch
        x1r_view = x1.rearrange("b (t p) h -> b p t h", p=128)
        x2r_view = x2.rearrange("b (t p) h -> b p t h", p=128)

        def s1_load(b, halves=False):
            """Claims + input DMAs for batch b (one DMA per tensor; batch 0
            loads column halves so the first transposes start sooner)."""
            xb1 = xp.tile([128, NT * H], F16, name="xb1", tag="xb1")
            xb2 = xp.tile([128, NT * H], F16, name="xb2", tag="xb2")
            for xb, xrv in ((xb1, x1r_view), (xb2, x2r_view)):
                nc.vector.memset(xb[0:1, NT * H - 1 :], 0.0)
                dst = xb[:].rearrange("p (t h) -> p t h", t=NT)
                if halves:
                    nc.sync.dma_start(dst[:, :, 0 : H // 2],
                                      xrv[b][:, :, 0 : H // 2])
                    nc.sync.dma_start(dst[:, :, H // 2 : H],
                                      xrv[b][:, :, H // 2 : H])
                else:
                    nc.sync.dma_start(dst, xrv[b])
            S = {"xb1": xb1, "xb2": xb2}
            S["xn1"] = [xb1[:, H * a : H * (a + 1)] for a in range(NT)]
            S["xn2"] = [xb2[:, H * a : H * (a + 1)] for a in range(NT)]
            return S

        def s2_xt(S):
            """PE transposes of x1/x2 (f32) + Pool rounding copies to f32r.

            Returns a list of 16 thunks (one per h-tile group) so the caller
            can interleave them with output-producing groups."""
            x1T = [xtp.tile([128, L], F16, name="x1T", tag="x1T")
                   for _ in range(HT)]
            x2T = [xtp.tile([128, L], F16, name="x2T", tag="x2T")
                   for _ in range(HT)]
            S["x1T"], S["x2T"] = x1T, x2T
            S["xn_touch"] = None

            def group(src, dstT, c, first):
                def emit():
                    if first:
                        S["xn_touch"] = [touch(t) for t in
                                         (S["xn1"][0], S["xn2"][0])]
                    tt = psA.tile([128, L], F32, name="psA", tag="psA")
                    tth = tt[:].bitcast(F16)[:, 0:L]
                    k = None
                    for a in range(NT):
                        inst = nc.tensor.matmul(
                            tth[:, 128 * a : 128 * (a + 1)],
                            src[a][:, 128 * c : 128 * (c + 1)],
                            identb[:],
                            is_transpose=True,
                        )
                        if a == 0:
                            k = gate("psA", 3, inst)
                            add_dep_helper(inst.ins, S["xn_touch"][-1].ins,
                                           sync=False, reason="xn touch gate")
                    nc.scalar.copy(dstT[c][:], tth[:])
                    set_gate("psA", k, touch(dstT[c]))
                return emit

            thunks = []
            for src_key, dstT in (("xn1", x1T), ("xn2", x2T)):
                for c in range(HT):
                    thunks.append(group(S[src_key], dstT, c,
                                        first=not thunks))
            return thunks

        def stats(em, probs, rz):
            """Masked softmax stats straight from the PSUM logits tile."""
            negmax = st.tile([128, 1], F32, name="negmax", tag="negmax")
            nc.vector.reduce_max(negmax[:], em[:], axis=AX, negate=True)
            z = st.tile([128, 1], F32, name="z", tag="z")
            nc.scalar.activation(probs[:], em[:], Exp, bias=negmax[:],
                                 accum_out=z[:])
            gates["last_probs_touch"] = touch(probs)
            nc.vector.reciprocal(rz[:], z[:])

        def s3_e12(S, b):
            """e12 logits, clean copy, +m2 rank-1, orientation-1 stats.

            Returns 4 thunks (one per i-tile)."""
            m2row = m2b[:, L * b : L * (b + 1)]
            e12c = [ecp.tile([128, L], F32, name="e12c", tag="e12c")
                    for _ in range(NT)]
            p12 = [pp.tile([128, L], F16, name="p12", tag="p12")
                   for _ in range(NT)]
            rz1 = [st.tile([128, 1], F32, name="rz1", tag="rz1")
                   for _ in range(NT)]
            S["e12c"], S["e12c_touch"] = e12c, []
            S["p12"], S["rz1"] = p12, rz1

            def group(a):
                def emit():
                    pe = psA.tile([128, L], F32, name="psA", tag="psA")
                    k = None
                    for c in range(HT):
                        inst = nc.tensor.matmul(
                            pe[:],
                            S["x1T"][c][:, 128 * a : 128 * (a + 1)],
                            S["x2T"][c][:],
                            start=(c == 0),
                            stop=(c == HT - 1),
                        )
                        if c == 0:
                            k = gate("psA", 3, inst)
                    nc.vector.tensor_copy(e12c[a][:], pe[:])
                    S["e12c_touch"].append(touch(e12c[a]))
                    nc.tensor.matmul(pe[:], onesb[:1, :], m2row, start=False,
                                     stop=True, skip_group_check=True)
                    stats(pe, p12[a], rz1[a])
                    set_gate("psA", k, gates["last_probs_touch"])
                return emit

            return [group(a) for a in range(NT)]

        def s4_e21(S, b):
            """e21 = transpose(e12 clean) + m1 rank-1, orientation-2 stats."""
            m1row = m1b[:, L * b : L * (b + 1)]
            p21 = [pp.tile([128, L], F16, name="p21", tag="p21")
                   for _ in range(NT)]
            rz2 = [st.tile([128, 1], F32, name="rz2", tag="rz2")
                   for _ in range(NT)]
            for c in range(NT):
                pe2 = psA.tile([128, L], F32, name="psA", tag="psA")
                inst = nc.tensor.matmul(pe2[:], onesb[:1, :], m1row,
                                        start=True, stop=True)
                k = gate("psA", 3, inst)
                add_dep_helper(inst.ins, S["e12c_touch"][-1].ins,
                               sync=False, reason="e12c touch gate")
                for a in range(NT):
                    nc.tensor.matmul(
                        pe2[:, 128 * a : 128 * (a + 1)],
                        S["e12c"][a][:, 128 * c : 128 * (c + 1)],
                        ident[:],
                        is_transpose=True,
                        start=False,
                        stop=True,
                        skip_group_check=True,
                    )
                stats(pe2, p21[c], rz2[c])
                set_gate("psA", k, gates["last_probs_touch"])
            S["p21"], S["rz2"] = p21, rz2

        def s5_pt(S, which):
            """bf16 PE transpose of probs -> contraction dim on partitions."""
            srcp = S[which]
            dstT = [ptp.tile([128, L], F16, name=which + "T", tag=which + "T")
                    for _ in range(NT)]
            for c in range(NT):
                tt = psTP.tile([128, L], F32, name="psTP", tag="psTP")
                ttb = tt[:].bitcast(F16)[:, 0:L]
                k = None
                for a in range(NT):
                    inst = nc.tensor.matmul(
                        ttb[:, 128 * a : 128 * (a + 1)],
                        srcp[a][:, 128 * c : 128 * (c + 1)],
                        identb[:],
                        is_transpose=True,
                    )
                    if a == 0:
                        k = gate("psTP", 2, inst)
                nc.scalar.copy(dstT[c][:], ttb[:])
                set_gate("psTP", k, touch(dstT[c]))
            S[which + "T"] = dstT

        def s6_stage2(S, b, which):
            """tilde = probsT^T @ x, normalize, enhance, DMA out (bf16).

            Returns 4 thunks (one per 128-row output tile)."""
            if which == "p12":
                pT, vals, xnat, rzs, y = S["p12T"], S["xn2"], S["xn1"], S["rz1"], y1
            else:
                pT, vals, xnat, rzs, y = S["p21T"], S["xn1"], S["xn2"], S["rz2"], y2

            yv = y.rearrange("b (q two p) c -> b q p two c", two=2, p=128)

            def group(q):
                def emit():
                    # two 128-row tiles per SBUF tile -> one merged DMA
                    ys2 = yp.tile([128, 2 * 4 * H], F16, name="ys", tag="ys")
                    nc.vector.memset(ys2[0:1, 0:1], 0.0)
                    for half in range(2):
                        a = 2 * q + half
                        ys = ys2[:, 4 * H * half : 4 * H * (half + 1)]
                        nc.gpsimd.tensor_copy(ys[:, 0:H], xnat[a][:])
                        for n in range(2):
                            pt = psB.tile([128, 512], F32, name="psB", tag="psB")
                            k = None
                            for c in range(NT):
                                inst = nc.tensor.matmul(
                                    pt[:],
                                    pT[c][:, 128 * a : 128 * (a + 1)],
                                    vals[c][:, 512 * n : 512 * (n + 1)],
                                    start=(c == 0),
                                    stop=(c == NT - 1),
                                )
                                if c == 0:
                                    k = gate("psB", 2, inst)
                            cols = slice(H + 512 * n, H + 512 * (n + 1))
                            if n == 0:
                                nc.scalar.activation(ys[:, cols], pt[:], Copy,
                                                     scale=rzs[a][:])
                            else:
                                nc.vector.tensor_scalar_mul(ys[:, cols], pt[:],
                                                            rzs[a][:])
                            set_gate("psB", k, touch(ys[:, cols]))
                        nc.vector.tensor_sub(ys[:, 2 * H : 3 * H], ys[:, 0:H],
                                             ys[:, H : 2 * H])
                        nc.gpsimd.tensor_mul(ys[:, 3 * H : 4 * H], ys[:, 0:H],
                                             ys[:, H : 2 * H])
                    nc.sync.dma_start(
                        yv[b, q],
                        ys2[:].rearrange("p (two c) -> p two c", two=2))
                return emit

            return [group(q) for q in range(NT // 2)]

        # -------- software-pipelined batch loop --------
        # PE order per iteration (b >= 1):
        #   pT transposes or1(b-1); then the 16 x-transpose groups of b
        #   interleaved with the 4 or1(b-1) output tiles; then pT or2(b-1);
        #   then the 4 e12 groups of b interleaved with the 4 or2(b-1)
        #   output tiles; then e21(b).
        # Outputs materialize evenly through the iteration, keeping the DMA
        # pipe fed, while the logits of b cover the softmax-stats latency of
        # b.  Input DMAs for b are queued at iteration start.
        def interleave(work, outs):
            """Merge work/outs thunk lists, spreading outs evenly."""
            if not outs:
                for w in work:
                    w()
                return
            stride = max(1, len(work) // len(outs))
            wi = 0
            for o in outs:
                for w in work[wi : wi + stride]:
                    w()
                wi += stride
                o()
            for w in work[wi:]:
                w()

        # Iteration k (steady state):
        #   s5b(k-1); {out2(k-1) x s2(k)}; loads(k+1); s3(k); s4(k);
        #   s5a(k); out1(k)
        # Orientation-2 outputs of k-1 stream while the x-transposes of k
        # run; the input DMAs for k+1 transfer during the (output-free)
        # logits phase; orientation-1 outputs of k stream at iteration end.
        states = {}
        held = []  # deferred output thunks from the previous iteration
        for b in range(BPC):
            if b == 0:
                S = states[0] = s1_load(0, halves=True)
                # warm the PE's view of the mask rows before their first use
                nc.tensor.matmul(scratch[0:32, 0:1],
                                 m1b[0:1, 0:32], onesb[0:1, 0:1],
                                 start=True, stop=True)
                nc.tensor.matmul(scratch[0:32, 0:1],
                                 m2b[0:1, 0:32], onesb[0:1, 0:1],
                                 start=True, stop=True)
                interleave(s2_xt(S), [])
            else:
                S = states[b]
                if b + 1 < BPC:
                    states[b + 1] = s1_load(b + 1)
                PS = states[b - 1]
                s5_pt(PS, "p21")
                out2 = s6_stage2(PS, b - 1, "p21")
                interleave(s2_xt(S), out2)
            if b == 0:
                states[1] = s1_load(1)
            # stream a deferred output from b-1 through the (otherwise
            # output-free) logits phase
            interleave(s3_e12(S, b), held)
            held = []
            if b == 0:
                # batch 0: get the first outputs moving before e21
                s5_pt(S, "p12")
                out1 = s6_stage2(S, b, "p12")
                out1[0]()
                s4_e21(S, b)
                held = out1[1:]
            elif b < BPC - 1:
                s4_e21(S, b)
                s5_pt(S, "p12")
                out1 = s6_stage2(S, b, "p12")
                out1[0]()
                held = out1[1:]
            else:
                # last batch: interleave the final orientation-2 work into
                # the orientation-1 outputs so all four closing DMAs stream
                # while PE finishes
                s4_e21(S, b)
                s5_pt(S, "p12")
                out1 = s6_stage2(S, b, "p12")
                out1[0]()
                s5_pt(S, "p21")
                out2 = s6_stage2(S, b, "p21")
                out1[1]()
                for o in out2:
                    o()
    if not nc.is_finalized():
        nc.finalize()
    return nc


NEGH = np.float16(-6.0e4)  # effective -inf for fp16 mask rows


def make_core_inputs(x1, l1, x2, l2):
    """Build the in_map for one core's shard (BPC batches)."""
    ar = np.arange(L, dtype=np.int32)
    m1 = np.where(ar[None, :] >= np.asarray(l1)[:, None], NEGH, np.float16(0))
    m2 = np.where(ar[None, :] >= np.asarray(l2)[:, None], NEGH, np.float16(0))
    return {"x1": np.asarray(x1, np.float16),
            "x2": np.asarray(x2, np.float16),
            "m1": m1.astype(np.float16), "m2": m2.astype(np.float16)}


def kernel(x1_bar, seq_lengths1, x2_bar, seq_lengths2):
    x1_bar = np.ascontiguousarray(x1_bar, dtype=np.float32)
    x2_bar = np.ascontiguousarray(x2_bar, dtype=np.float32)

    if "nc" not in _NC_CACHE:
        _NC_CACHE["nc"] = build_nc()
    nc = _NC_CACHE["nc"]

    in_maps = []
    for c in range(NCORES):
        s = slice(c * BPC, (c + 1) * BPC)
        in_maps.append(make_core_inputs(
            x1_bar[s], np.asarray(seq_lengths1)[s],
            x2_bar[s], np.asarray(seq_lengths2)[s]))

    res = run_bass_kernel_spmd(nc, in_maps, core_ids=list(range(NCORES)))
    y1 = np.concatenate([np.asarray(r["y1"], np.float32) for r in res.results],
                        axis=0)
    y2 = np.concatenate([np.asarray(r["y2"], np.float32) for r in res.results],
                        axis=0)
    return y1, y2


# revision 58
# speedup vs baseline: 1.0615x; 1.0615x over previous
"""Trainium2 Bass kernel for nn_LocalInferenceModeling (cross-attention enhance).

Reference computation (per batch b):
    e = x1 @ x2^T                                  [L, L]
    a12 = softmax_j(e + m2[j]);  x1t = a12 @ x2    [L, H]
    a21 = softmax_i(e^T + m1[i]); x2t = a21 @ x1   [L, H]
    y1 = concat([x1, x1t, x1 - x1t, x1 * x1t], -1) [L, 4H]
    y2 = concat([x2, x2t, x2 - x2t, x2 * x2t], -1)

Sharding: batch dim B=32 split across 8 NeuronCores (4 batches/core),
no communication.  Masks (0 / -1e30 rows from seq_lengths) are computed
host-side and passed as extra inputs.

Per-core dataflow (per batch), fp16 end-to-end (inputs are converted to
fp16 host-side; the 2e-2 accuracy budget has ample room, measured ~6e-3):
  - one DMA per input tensor loads a whole fp16 batch into a [128, 4096]
    tile (partition p holds rows {128a+p} as column blocks a)
  - PE-transpose (fp16, 1 cyc/row) -> x1T, x2T [8x(128,512)] (h on
    partitions), copies to SBUF
  - e12 [i,j]: matmul(lhsT=x1T, rhs=x2T) accum over 8 h-tiles into f32
    PSUM; clean f32 copy to SBUF, then an fp16 rank-1 mask row
    (ones^T @ m2, mask value -6e4 since -1e30 overflows fp16)
  - e21 [j,i]: rank-1 m1 into a fresh PSUM bank, then accumulate f32 PE
    transposes of the clean e12 copy (exact, 5x cheaper than a second
    full matmul)
  - masked softmax read straight from PSUM: reduce_max(negate) ->
    Exp(bias=-max, accum_out=z) -> reciprocal; probs UNNORMALIZED fp16
    (1/z is applied after stage 2)
  - PE-transpose probs (fp16) -> p12T/p21T
  - stage 2: tilde = probsT^T @ x (fp16 x fp16, consuming the input
    tiles directly - no value copies), normalized by 1/z during the
    PSUM->SBUF copy straight into an fp16 [128, 2x4H] output tile along
    with x_bar / sub / mul slices
  - one fp16 DMA per 256 output rows; host converts back to fp32

The batch loop is software-pipelined at orientation granularity so the
in-order PE stream never stalls on the DVE/Act softmax-stats chain and
output DMAs are spread across each iteration to keep the DMA engines
(the binding resource: ~40 MB/core at 360 GB/s) continuously fed:
  iter k: s5b(k-1); loads(k+1); {out2(k-1) x xT-transposes(k)};
          {held out1(k-1) x e12(k)}; e21(k); s5a(k); out1(k) [1 held]
with a special first iteration (outputs before e21) and last iteration
(orientation-2 interleaved into the closing outputs).

Walrus constraints honored: f32r operands must be rounding-copies (not
bitcasts), matmul operands share a transfer-type family, and the Pool
engine never touches PSUM.
"""

import sys

import numpy as np

sys.path.insert(0, "/opt/trn_rl_repo")

from contextlib import ExitStack

import concourse.bass as bass
import concourse.bacc as bacc
import concourse.mybir as mybir
from concourse import masks
from concourse.bass_utils import run_bass_kernel_spmd
from concourse.tile import TileContext

B, L, H = 32, 512, 1024
NCORES = 8
BPC = B // NCORES  # batches per core
NEG = np.float32(-1.0e30)

F32 = mybir.dt.float32
F32R = mybir.dt.float32r
BF16 = mybir.dt.bfloat16
F16 = mybir.dt.float16

NT = L // 128  # 4 partition tiles per L
HT = H // 128  # 8 partition tiles per H
Exp = mybir.ActivationFunctionType.Exp
Copy = mybir.ActivationFunctionType.Copy
AX = mybir.AxisListType.X

_NC_CACHE = {}


def build_nc():
    nc = bacc.Bacc(None, target_bir_lowering=False)
    x1 = nc.dram_tensor("x1", [BPC, L, H], F16, kind="ExternalInput")
    x2 = nc.dram_tensor("x2", [BPC, L, H], F16, kind="ExternalInput")
    m1 = nc.dram_tensor("m1", [BPC, L], F16, kind="ExternalInput")
    m2 = nc.dram_tensor("m2", [BPC, L], F16, kind="ExternalInput")
    y1 = nc.dram_tensor("y1", [BPC, L, 4 * H], F16, kind="ExternalOutput")
    y2 = nc.dram_tensor("y2", [BPC, L, 4 * H], F16, kind="ExternalOutput")

    with TileContext(nc) as tc, ExitStack() as ctx:
        from concourse.tile import add_dep_helper

        const = ctx.enter_context(tc.tile_pool(name="const", bufs=1))
        ident = const.tile([128, 128], F32)
        masks.make_identity(nc, ident[:])
        ones = const.tile([1, 128], F32)
        nc.vector.memset(ones[:], 1.0)
        # fp16 identity / ones for the fp16-rate transposes and rank-1s
        identb = const.tile([128, 128], F16, name="identb")
        nc.gpsimd.tensor_copy(identb[:], ident[:])
        onesb = const.tile([1, 128], F16, name="onesb")
        nc.vector.memset(onesb[:], 1.0)

        xp = ctx.enter_context(tc.tile_pool(name="xp", bufs=3))
        xtp = ctx.enter_context(tc.tile_pool(name="xtp", bufs=HT))
        ecp = ctx.enter_context(tc.tile_pool(name="ecp", bufs=NT + 1))
        pp = ctx.enter_context(tc.tile_pool(name="pp", bufs=NT + 1))
        ptp = ctx.enter_context(tc.tile_pool(name="ptp", bufs=NT + 1))
        st = ctx.enter_context(tc.tile_pool(name="st", bufs=8 * NT))
        yp = ctx.enter_context(tc.tile_pool(name="yp", bufs=3))
        mrp = ctx.enter_context(tc.tile_pool(name="mrp", bufs=1))
        psA = ctx.enter_context(tc.tile_pool(name="psA", bufs=3, space="PSUM"))
        psTP = ctx.enter_context(tc.tile_pool(name="psTP", bufs=2, space="PSUM"))
        psB = ctx.enter_context(tc.tile_pool(name="psB", bufs=2, space="PSUM"))
        psS = ctx.enter_context(tc.tile_pool(name="psS", bufs=1, space="PSUM"))
        scratch = psS.tile([32, 32], F32, name="scratch", tag="scratch")

        gates = {"psA": [], "psTP": [], "psB": []}

        def touch(ap):
            # Tiny PE transpose reading `ap` so the PE engine observes the
            # producer's sem tick; real matmuls then carry at most one sync
            # wait (walrus can encode only one on self-loading matmuls).
            a32 = ap[0:32, 0:32]
            if a32.dtype != F32:
                if mybir.dt.size(a32.dtype) == 2:
                    a32 = ap[0:32, 0:64].bitcast(F32)
                else:
                    a32 = a32.bitcast(F32)
            with tc.high_priority(offset=200):
                return nc.tensor.transpose(scratch[:], a32, ident[0:32, 0:32])

        def gate(tag, bufs, first_inst):
            # Order the group's first PE write after the touch that observed
            # the release of the slot it reuses (bufs groups back).
            hist = gates[tag]
            k = len(hist)
            if k >= bufs and hist[k - bufs] is not None:
                add_dep_helper(first_inst.ins, hist[k - bufs].ins, sync=False,
                               reason="psum slot gate")
            hist.append(None)  # placeholder until release touch known
            return k

        def set_gate(tag, k, tinst):
            gates[tag][k] = tinst

        touch(ident)
        nc.tensor.matmul(scratch[0:32, 0:1], ones[0:1, 0:32], ones[0:1, 0:1],
                         start=True, stop=True)

        # fp16 mask rows for the rank-1 mask matmuls
        m1b = mrp.tile([1, BPC * L], F16, name="m1b", tag="m1b")
        m2b = mrp.tile([1, BPC * L], F16, name="m2b", tag="m2b")
        nc.sync.dma_start(m1b[:1, :], m1.rearrange("b l -> (b l)")[None, :])
        nc.sync.dma_start(m2b[:1, :], m2.rearrange("b l -> (b l)")[None, :])

        # [BPC, 128, NT*H] view: partition p of batch b holds rows {128a+p}
        # as column blocks a — one DMA loads a whole batch
        x1r_view = x1.rearrange("b (t p) h -> b p t h", p=128)
        x2r_view = x2.rearrange("b (t p) h -> b p t h", p=128)

        def s1_load(b, halves=False):
            """Claims + input DMAs for batch b (one DMA per tensor; batch 0
            loads column halves so the first transposes start sooner)."""
            xb1 = xp.tile([128, NT * H], F16, name="xb1", tag="xb1")
            xb2 = xp.tile([128, NT * H], F16, name="xb2", tag="xb2")
            for xb, xrv in ((xb1, x1r_view), (xb2, x2r_view)):
                nc.vector.memset(xb[0:1, NT * H - 1 :], 0.0)
                dst = xb[:].rearrange("p (t h) -> p t h", t=NT)
                if halves:
                    nc.sync.dma_start(dst[:, :, 0 : H // 2],
                                      xrv[b][:, :, 0 : H // 2])
                    nc.sync.dma_start(dst[:, :, H // 2 : H],
                                      xrv[b][:, :, H // 2 : H])
                else:
                    nc.sync.dma_start(dst, xrv[b])
            S = {"xb1": xb1, "xb2": xb2}
            S["xn1"] = [xb1[:, H * a : H * (a + 1)] for a in range(NT)]
            S["xn2"] = [xb2[:, H * a : H * (a + 1)] for a in range(NT)]
            return S

        def s2_xt(S):
            """PE transposes of x1/x2 (f32) + Pool rounding copies to f32r.

            Returns a list of 16 thunks (one per h-tile group) so the caller
            can interleave them with output-producing groups."""
            x1T = [xtp.tile([128, L], F16, name="x1T", tag="x1T")
                   for _ in range(HT)]
            x2T = [xtp.tile([128, L], F16, name="x2T", tag="x2T")
                   for _ in range(HT)]
            S["x1T"], S["x2T"] = x1T, x2T
            S["xn_touch"] = None

            def group(src, dstT, c, first):
                def emit():
                    if first:
                        S["xn_touch"] = [touch(t) for t in
                                         (S["xn1"][0], S["xn2"][0])]
                    tt = psA.tile([128, L], F32, name="psA", tag="psA")
                    tth = tt[:].bitcast(F16)[:, 0:L]
                    k = None
                    for a in range(NT):
                        inst = nc.tensor.matmul(
                            tth[:, 128 * a : 128 * (a + 1)],
                            src[a][:, 128 * c : 128 * (c + 1)],
                            identb[:],
                            is_transpose=True,
                        )
                        if a == 0:
                            k = gate("psA", 3, inst)
                            add_dep_helper(inst.ins, S["xn_touch"][-1].ins,
                                           sync=False, reason="xn touch gate")
                    nc.scalar.copy(dstT[c][:], tth[:])
                    set_gate("psA", k, touch(dstT[c]))
                return emit

            thunks = []
            for src_key, dstT in (("xn1", x1T), ("xn2", x2T)):
                for c in range(HT):
                    thunks.append(group(S[src_key], dstT, c,
                                        first=not thunks))
            return thunks

        def stats(em, probs, rz):
            """Masked softmax stats straight from the PSUM logits tile."""
            negmax = st.tile([128, 1], F32, name="negmax", tag="negmax")
            nc.vector.reduce_max(negmax[:], em[:], axis=AX, negate=True)
            z = st.tile([128, 1], F32, name="z", tag="z")
            nc.scalar.activation(probs[:], em[:], Exp, bias=negmax[:],
                                 accum_out=z[:])
            gates["last_probs_touch"] = touch(probs)
            nc.vector.reciprocal(rz[:], z[:])

        def s3_e12(S, b):
            """e12 logits, clean copy, +m2 rank-1, orientation-1 stats.

            Returns 4 thunks (one per i-tile)."""
            m2row = m2b[:, L * b : L * (b + 1)]
            e12c = [ecp.tile([128, L], F32, name="e12c", tag="e12c")
                    for _ in range(NT)]
            p12 = [pp.tile([128, L], F16, name="p12", tag="p12")
                   for _ in range(NT)]
            rz1 = [st.tile([128, 1], F32, name="rz1", tag="rz1")
                   for _ in range(NT)]
            S["e12c"], S["e12c_touch"] = e12c, []
            S["p12"], S["rz1"] = p12, rz1

            def group(a):
                def emit():
                    pe = psA.tile([128, L], F32, name="psA", tag="psA")
                    k = None
                    for c in range(HT):
                        inst = nc.tensor.matmul(
                            pe[:],
                            S["x1T"][c][:, 128 * a : 128 * (a + 1)],
                            S["x2T"][c][:],
                            start=(c == 0),
                            stop=(c == HT - 1),
                        )
                        if c == 0:
                            k = gate("psA", 3, inst)
                    nc.vector.tensor_copy(e12c[a][:], pe[:])
                    S["e12c_touch"].append(touch(e12c[a]))
                    nc.tensor.matmul(pe[:], onesb[:1, :], m2row, start=False,
                                     stop=True, skip_group_check=True)
                    stats(pe, p12[a], rz1[a])
                    set_gate("psA", k, gates["last_probs_touch"])
                return emit

            return [group(a) for a in range(NT)]

        def s4_e21(S, b):
            """e21 = transpose(e12 clean) + m1 rank-1, orientation-2 stats."""
            m1row = m1b[:, L * b : L * (b + 1)]
            p21 = [pp.tile([128, L], F16, name="p21", tag="p21")
                   for _ in range(NT)]
            rz2 = [st.tile([128, 1], F32, name="rz2", tag="rz2")
                   for _ in range(NT)]
            for c in range(NT):
                pe2 = psA.tile([128, L], F32, name="psA", tag="psA")
                inst = nc.tensor.matmul(pe2[:], onesb[:1, :], m1row,
                                        start=True, stop=True)
                k = gate("psA", 3, inst)
                add_dep_helper(inst.ins, S["e12c_touch"][-1].ins,
                               sync=False, reason="e12c touch gate")
                for a in range(NT):
                    nc.tensor.matmul(
                        pe2[:, 128 * a : 128 * (a + 1)],
                        S["e12c"][a][:, 128 * c : 128 * (c + 1)],
                        ident[:],
                        is_transpose=True,
                        start=False,
                        stop=True,
                        skip_group_check=True,
                    )
                stats(pe2, p21[c], rz2[c])
                set_gate("psA", k, gates["last_probs_touch"])
            S["p21"], S["rz2"] = p21, rz2

        def s5_pt(S, which):
            """bf16 PE transpose of probs -> contraction dim on partitions."""
            srcp = S[which]
            dstT = [ptp.tile([128, L], F16, name=which + "T", tag=which + "T")
                    for _ in range(NT)]
            for c in range(NT):
                tt = psTP.tile([128, L], F32, name="psTP", tag="psTP")
                ttb = tt[:].bitcast(F16)[:, 0:L]
                k = None
                for a in range(NT):
                    inst = nc.tensor.matmul(
                        ttb[:, 128 * a : 128 * (a + 1)],
                        srcp[a][:, 128 * c : 128 * (c + 1)],
                        identb[:],
                        is_transpose=True,
                    )
                    if a == 0:
                        k = gate("psTP", 2, inst)
                nc.scalar.copy(dstT[c][:], ttb[:])
                set_gate("psTP", k, touch(dstT[c]))
            S[which + "T"] = dstT

        def s6_stage2(S, b, which):
            """tilde = probsT^T @ x, normalize, enhance, DMA out (bf16).

            Returns 4 thunks (one per 128-row output tile)."""
            if which == "p12":
                pT, vals, xnat, rzs, y = S["p12T"], S["xn2"], S["xn1"], S["rz1"], y1
            else:
                pT, vals, xnat, rzs, y = S["p21T"], S["xn1"], S["xn2"], S["rz2"], y2

            yv = y.rearrange("b (q two p) c -> b q p two c", two=2, p=128)

            def group(q):
                def emit():
                    # two 128-row tiles per SBUF tile -> one merged DMA
                    ys2 = yp.tile([128, 2 * 4 * H], F16, name="ys", tag="ys")
                    nc.vector.memset(ys2[0:1, 0:1], 0.0)
                    for half in range(2):
                        a = 2 * q + half
                        ys = ys2[:, 4 * H * half : 4 * H * (half + 1)]
                        nc.gpsimd.tensor_copy(ys[:, 0:H], xnat[a][:])
                        for n in range(2):
                            pt = psB.tile([128, 512], F32, name="psB", tag="psB")
                            k = None
                            for c in range(NT):
                                inst = nc.tensor.matmul(
                                    pt[:],
                                    pT[c][:, 128 * a : 128 * (a + 1)],
                                    vals[c][:, 512 * n : 512 * (n + 1)],
                                    start=(c == 0),
                                    stop=(c == NT - 1),
                                )
                                if c == 0:
                                    k = gate("psB", 2, inst)
                            cols = slice(H + 512 * n, H + 512 * (n + 1))
                            if n == 0:
                                nc.scalar.activation(ys[:, cols], pt[:], Copy,
                                                     scale=rzs[a][:])
                            else:
                                nc.vector.tensor_scalar_mul(ys[:, cols], pt[:],
                                                            rzs[a][:])
                            set_gate("psB", k, touch(ys[:, cols]))
                        nc.vector.tensor_sub(ys[:, 2 * H : 3 * H], ys[:, 0:H],
                                             ys[:, H : 2 * H])
                        nc.gpsimd.tensor_mul(ys[:, 3 * H : 4 * H], ys[:, 0:H],
                                             ys[:, H : 2 * H])
                    nc.sync.dma_start(
                        yv[b, q],
                        ys2[:].rearrange("p (two c) -> p two c", two=2))
                return emit

            return [group(q) for q in range(NT // 2)]

        # -------- software-pipelined batch loop --------
        # PE order per iteration (b >= 1):
        #   pT transposes or1(b-1); then the 16 x-transpose groups of b
        #   interleaved with the 4 or1(b-1) output tiles; then pT or2(b-1);
        #   then the 4 e12 groups of b interleaved with the 4 or2(b-1)
        #   output tiles; then e21(b).
        # Outputs materialize evenly through the iteration, keeping the DMA
        # pipe fed, while the logits of b cover the softmax-stats latency of
        # b.  Input DMAs for b are queued at iteration start.
        def interleave(work, outs):
            """Merge work/outs thunk lists, spreading outs evenly."""
            if not outs:
                for w in work:
                    w()
                return
            stride = max(1, len(work) // len(outs))
            wi = 0
            for o in outs:
                for w in work[wi : wi + stride]:
                    w()
                wi += stride
                o()
            for w in work[wi:]:
                w()

        # Iteration k (steady state):
        #   s5b(k-1); {out2(k-1) x s2(k)}; loads(k+1); s3(k); s4(k);
        #   s5a(k); out1(k)
        # Orientation-2 outputs of k-1 stream while the x-transposes of k
        # run; the input DMAs for k+1 transfer during the (output-free)
        # logits phase; orientation-1 outputs of k stream at iteration end.
        states = {}
        held = []  # deferred output thunks from the previous iteration
        for b in range(BPC):
            if b == 0:
                S = states[0] = s1_load(0, halves=True)
                # warm the PE's view of the mask rows before their first use
                nc.tensor.matmul(scratch[0:32, 0:1],
                                 m1b[0:1, 0:32], onesb[0:1, 0:1],
                                 start=True, stop=True)
                nc.tensor.matmul(scratch[0:32, 0:1],
                                 m2b[0:1, 0:32], onesb[0:1, 0:1],
                                 start=True, stop=True)
                interleave(s2_xt(S), [])
            else:
                S = states[b]
                if b + 1 < BPC:
                    states[b + 1] = s1_load(b + 1)
                PS = states[b - 1]
                s5_pt(PS, "p21")
                out2 = s6_stage2(PS, b - 1, "p21")
                interleave(s2_xt(S), out2)
            if b == 0:
                states[1] = s1_load(1)
            # stream a deferred output from b-1 through the (otherwise
            # output-free) logits phase
            interleave(s3_e12(S, b), held)
            held = []
            if b == 0:
                # batch 0: get the first outputs moving before e21
                s5_pt(S, "p12")
                out1 = s6_stage2(S, b, "p12")
                out1[0]()
                s4_e21(S, b)
                held = out1[1:]
            elif b < BPC - 1:
                s4_e21(S, b)
                s5_pt(S, "p12")
                out1 = s6_stage2(S, b, "p12")
                out1[0]()
                held = out1[1:]
            else:
                # last batch: interleave the final orientation-2 work into
                # the orientation-1 outputs so all four closing DMAs stream
                # while PE finishes
                s4_e21(S, b)
                s5_pt(S, "p12")
                out1 = s6_stage2(S, b, "p12")
                out1[0]()
                s5_pt(S, "p21")
                out2 = s6_stage2(S, b, "p21")
                out1[1]()
                for o in out2:
                    o()
    if not nc.is_finalized():
        nc.finalize()
    return nc


NEGH = np.float16(-6.0e4)  # effective -inf for fp16 mask rows


def make_core_inputs(x1, l1, x2, l2):
    """Build the in_map for one core's shard (BPC batches)."""
    ar = np.arange(L, dtype=np.int32)
    m1 = np.where(ar[None, :] >= np.asarray(l1)[:, None], NEGH, np.float16(0))
    m2 = np.where(ar[None, :] >= np.asarray(l2)[:, None], NEGH, np.float16(0))
    return {"x1": np.asarray(x1, np.float16),
            "x2": np.asarray(x2, np.float16),
            "m1": m1.astype(np.float16), "m2": m2.astype(np.float16)}


def kernel(x1_bar, seq_lengths1, x2_bar, seq_lengths2):
    x1_bar = np.ascontiguousarray(x1_bar, dtype=np.float32)
    x2_bar = np.ascontiguousarray(x2_bar, dtype=np.float32)

    if "nc" not in _NC_CACHE:
        _NC_CACHE["nc"] = build_nc()
    nc = _NC_CACHE["nc"]

    in_maps = []
    for c in range(NCORES):
        s = slice(c * BPC, (c + 1) * BPC)
        in_maps.append(make_core_inputs(
            x1_bar[s], np.asarray(seq_lengths1)[s],
            x2_bar[s], np.asarray(seq_lengths2)[s]))

    res = run_bass_kernel_spmd(nc, in_maps, core_ids=list(range(NCORES)))
    y1 = np.concatenate([np.asarray(r["y1"], np.float32) for r in res.results],
                        axis=0)
    y2 = np.concatenate([np.asarray(r["y2"], np.float32) for r in res.results],
                        axis=0)
    return y1, y2


# revision 59
# speedup vs baseline: 1.1508x; 1.0842x over previous
"""Trainium2 Bass kernel for nn_LocalInferenceModeling (cross-attention enhance).

Reference computation (per batch b):
    e = x1 @ x2^T                                  [L, L]
    a12 = softmax_j(e + m2[j]);  x1t = a12 @ x2    [L, H]
    a21 = softmax_i(e^T + m1[i]); x2t = a21 @ x1   [L, H]
    y1 = concat([x1, x1t, x1 - x1t, x1 * x1t], -1) [L, 4H]
    y2 = concat([x2, x2t, x2 - x2t, x2 * x2t], -1)

Sharding: batch dim B=32 split across 8 NeuronCores (4 batches/core),
no communication.  Masks (0 / -1e30 rows from seq_lengths) are computed
host-side and passed as extra inputs.

Per-core dataflow (per batch), fp16 end-to-end (inputs are converted to
fp16 host-side; the 2e-2 accuracy budget has ample room, measured ~6e-3):
  - one DMA per input tensor loads a whole fp16 batch into a [128, 4096]
    tile (partition p holds rows {128a+p} as column blocks a)
  - PE-transpose (fp16, 1 cyc/row) -> x1T, x2T [8x(128,512)] (h on
    partitions), copies to SBUF
  - e12 [i,j]: matmul(lhsT=x1T, rhs=x2T) accum over 8 h-tiles into f32
    PSUM; clean f32 copy to SBUF, then an fp16 rank-1 mask row
    (ones^T @ m2, mask value -6e4 since -1e30 overflows fp16)
  - e21 [j,i]: rank-1 m1 into a fresh PSUM bank, then accumulate f32 PE
    transposes of the clean e12 copy (exact, 5x cheaper than a second
    full matmul)
  - masked softmax read straight from PSUM: reduce_max(negate) ->
    Exp(bias=-max, accum_out=z) -> reciprocal; probs UNNORMALIZED fp16
    (1/z is applied after stage 2)
  - PE-transpose probs (fp16) -> p12T/p21T
  - stage 2: tilde = probsT^T @ x (fp16 x fp16, consuming the input
    tiles directly - no value copies), normalized by 1/z during the
    PSUM->SBUF copy straight into an fp16 [128, 2x4H] output tile along
    with x_bar / sub / mul slices
  - one fp16 DMA per 256 output rows; host converts back to fp32

The batch loop is software-pipelined at orientation granularity so the
in-order PE stream never stalls on the DVE/Act softmax-stats chain and
output DMAs are spread across each iteration to keep the DMA engines
(the binding resource: ~40 MB/core at 360 GB/s) continuously fed:
  iter k: s5b(k-1); loads(k+1); {out2(k-1) x xT-transposes(k)};
          {held out1(k-1) x e12(k)}; e21(k); s5a(k); out1(k) [1 held]
with a special first iteration (outputs before e21) and last iteration
(orientation-2 interleaved into the closing outputs).

Walrus constraints honored: f32r operands must be rounding-copies (not
bitcasts), matmul operands share a transfer-type family, and the Pool
engine never touches PSUM.
"""

import sys

import numpy as np

sys.path.insert(0, "/opt/trn_rl_repo")

from contextlib import ExitStack

import concourse.bass as bass
import concourse.bacc as bacc
import concourse.mybir as mybir
from concourse import masks
from concourse.bass_utils import run_bass_kernel_spmd
from concourse.tile import TileContext

B, L, H = 32, 512, 1024
NCORES = 8
BPC = B // NCORES  # batches per core
NEG = np.float32(-1.0e30)

F32 = mybir.dt.float32
F32R = mybir.dt.float32r
BF16 = mybir.dt.bfloat16
F16 = mybir.dt.float16

NT = L // 128  # 4 partition tiles per L
HT = H // 128  # 8 partition tiles per H
Exp = mybir.ActivationFunctionType.Exp
Copy = mybir.ActivationFunctionType.Copy
AX = mybir.AxisListType.X

_NC_CACHE = {}


def build_nc():
    nc = bacc.Bacc(None, target_bir_lowering=False)
    x1 = nc.dram_tensor("x1", [BPC, L, H], F16, kind="ExternalInput")
    x2 = nc.dram_tensor("x2", [BPC, L, H], F16, kind="ExternalInput")
    m1 = nc.dram_tensor("m1", [BPC, L], F16, kind="ExternalInput")
    m2 = nc.dram_tensor("m2", [BPC, L], F16, kind="ExternalInput")
    y1 = nc.dram_tensor("y1", [BPC, L, 4 * H], F16, kind="ExternalOutput")
    y2 = nc.dram_tensor("y2", [BPC, L, 4 * H], F16, kind="ExternalOutput")

    with TileContext(nc) as tc, ExitStack() as ctx:
        from concourse.tile import add_dep_helper

        const = ctx.enter_context(tc.tile_pool(name="const", bufs=1))
        ident = const.tile([128, 128], F32)
        masks.make_identity(nc, ident[:])
        ones = const.tile([1, 128], F32)
        nc.vector.memset(ones[:], 1.0)
        # fp16 identity / ones for the fp16-rate transposes and rank-1s
        identb = const.tile([128, 128], F16, name="identb")
        nc.gpsimd.tensor_copy(identb[:], ident[:])
        onesb = const.tile([1, 128], F16, name="onesb")
        nc.vector.memset(onesb[:], 1.0)

        xp = ctx.enter_context(tc.tile_pool(name="xp", bufs=3))
        xtp = ctx.enter_context(tc.tile_pool(name="xtp", bufs=HT))
        ecp = ctx.enter_context(tc.tile_pool(name="ecp", bufs=NT + 1))
        pp = ctx.enter_context(tc.tile_pool(name="pp", bufs=NT + 1))
        ptp = ctx.enter_context(tc.tile_pool(name="ptp", bufs=NT + 1))
        st = ctx.enter_context(tc.tile_pool(name="st", bufs=8 * NT))
        yp = ctx.enter_context(tc.tile_pool(name="yp", bufs=3))
        mrp = ctx.enter_context(tc.tile_pool(name="mrp", bufs=1))
        psA = ctx.enter_context(tc.tile_pool(name="psA", bufs=3, space="PSUM"))
        psTP = ctx.enter_context(tc.tile_pool(name="psTP", bufs=2, space="PSUM"))
        psB = ctx.enter_context(tc.tile_pool(name="psB", bufs=2, space="PSUM"))
        psS = ctx.enter_context(tc.tile_pool(name="psS", bufs=1, space="PSUM"))
        scratch = psS.tile([32, 32], F32, name="scratch", tag="scratch")

        gates = {"psA": [], "psTP": [], "psB": []}

        def touch(ap):
            # Tiny PE transpose reading `ap` so the PE engine observes the
            # producer's sem tick; real matmuls then carry at most one sync
            # wait (walrus can encode only one on self-loading matmuls).
            a32 = ap[0:32, 0:32]
            if a32.dtype != F32:
                if mybir.dt.size(a32.dtype) == 2:
                    a32 = ap[0:32, 0:64].bitcast(F32)
                else:
                    a32 = a32.bitcast(F32)
            with tc.high_priority(offset=200):
                return nc.tensor.transpose(scratch[:], a32, ident[0:32, 0:32])

        def gate(tag, bufs, first_inst):
            # Order the group's first PE write after the touch that observed
            # the release of the slot it reuses (bufs groups back).
            hist = gates[tag]
            k = len(hist)
            if k >= bufs and hist[k - bufs] is not None:
                add_dep_helper(first_inst.ins, hist[k - bufs].ins, sync=False,
                               reason="psum slot gate")
            hist.append(None)  # placeholder until release touch known
            return k

        def set_gate(tag, k, tinst):
            gates[tag][k] = tinst

        touch(ident)
        nc.tensor.matmul(scratch[0:32, 0:1], ones[0:1, 0:32], ones[0:1, 0:1],
                         start=True, stop=True)

        # fp16 mask rows for the rank-1 mask matmuls
        m1b = mrp.tile([1, BPC * L], F16, name="m1b", tag="m1b")
        m2b = mrp.tile([1, BPC * L], F16, name="m2b", tag="m2b")
        nc.sync.dma_start(m1b[:1, :], m1.rearrange("b l -> (b l)")[None, :])
        nc.sync.dma_start(m2b[:1, :], m2.rearrange("b l -> (b l)")[None, :])

        # [BPC, 128, NT*H] view: partition p of batch b holds rows {128a+p}
        # as column blocks a — one DMA loads a whole batch
        x1r_view = x1.rearrange("b (t p) h -> b p t h", p=128)
        x2r_view = x2.rearrange("b (t p) h -> b p t h", p=128)

        def s1_load(b, halves=False):
            """Claims + input DMAs for batch b (one DMA per tensor; batch 0
            loads column halves so the first transposes start sooner)."""
            xb1 = xp.tile([128, NT * H], F16, name="xb1", tag="xb1")
            xb2 = xp.tile([128, NT * H], F16, name="xb2", tag="xb2")
            for xb, xrv in ((xb1, x1r_view), (xb2, x2r_view)):
                nc.vector.memset(xb[0:1, NT * H - 1 :], 0.0)
                dst = xb[:].rearrange("p (t h) -> p t h", t=NT)
                if halves:
                    for q in range(4):
                        cs = slice(q * H // 4, (q + 1) * H // 4)
                        nc.sync.dma_start(dst[:, :, cs], xrv[b][:, :, cs])
                else:
                    nc.sync.dma_start(dst, xrv[b])
            S = {"xb1": xb1, "xb2": xb2}
            S["xn1"] = [xb1[:, H * a : H * (a + 1)] for a in range(NT)]
            S["xn2"] = [xb2[:, H * a : H * (a + 1)] for a in range(NT)]
            return S

        def s2_xt(S):
            """PE transposes of x1/x2 (f32) + Pool rounding copies to f32r.

            Returns a list of 16 thunks (one per h-tile group) so the caller
            can interleave them with output-producing groups."""
            x1T = [xtp.tile([128, L], F16, name="x1T", tag="x1T")
                   for _ in range(HT)]
            x2T = [xtp.tile([128, L], F16, name="x2T", tag="x2T")
                   for _ in range(HT)]
            S["x1T"], S["x2T"] = x1T, x2T
            S["xn_touch"] = None

            def group(src, dstT, c, first):
                def emit():
                    if first:
                        S["xn_touch"] = [touch(t) for t in
                                         (S["xn1"][0], S["xn2"][0])]
                    tt = psA.tile([128, L], F32, name="psA", tag="psA")
                    tth = tt[:].bitcast(F16)[:, 0:L]
                    k = None
                    for a in range(NT):
                        inst = nc.tensor.matmul(
                            tth[:, 128 * a : 128 * (a + 1)],
                            src[a][:, 128 * c : 128 * (c + 1)],
                            identb[:],
                            is_transpose=True,
                        )
                        if a == 0:
                            k = gate("psA", 3, inst)
                            add_dep_helper(inst.ins, S["xn_touch"][-1].ins,
                                           sync=False, reason="xn touch gate")
                    if c % 2 == 0:
                        nc.scalar.copy(dstT[c][:], tth[:])
                    else:
                        nc.vector.tensor_copy(dstT[c][:], tth[:])
                    set_gate("psA", k, touch(dstT[c]))
                return emit

            thunks = []
            for src_key, dstT in (("xn1", x1T), ("xn2", x2T)):
                for c in range(HT):
                    thunks.append(group(S[src_key], dstT, c,
                                        first=not thunks))
            return thunks

        def stats(em, probs, rz):
            """Masked softmax stats straight from the PSUM logits tile."""
            negmax = st.tile([128, 1], F32, name="negmax", tag="negmax")
            nc.vector.reduce_max(negmax[:], em[:], axis=AX, negate=True)
            z = st.tile([128, 1], F32, name="z", tag="z")
            nc.scalar.activation(probs[:], em[:], Exp, bias=negmax[:],
                                 accum_out=z[:])
            gates["last_probs_touch"] = touch(probs)
            nc.vector.reciprocal(rz[:], z[:])

        def s3_e12(S, b):
            """e12 logits, clean copy, +m2 rank-1, orientation-1 stats.

            Returns 4 thunks (one per i-tile)."""
            m2row = m2b[:, L * b : L * (b + 1)]
            e12c = [ecp.tile([128, L], F32, name="e12c", tag="e12c")
                    for _ in range(NT)]
            p12 = [pp.tile([128, L], F16, name="p12", tag="p12")
                   for _ in range(NT)]
            rz1 = [st.tile([128, 1], F32, name="rz1", tag="rz1")
                   for _ in range(NT)]
            S["e12c"], S["e12c_touch"] = e12c, []
            S["p12"], S["rz1"] = p12, rz1

            def group(a):
                def emit():
                    pe = psA.tile([128, L], F32, name="psA", tag="psA")
                    k = None
                    for c in range(HT):
                        inst = nc.tensor.matmul(
                            pe[:],
                            S["x1T"][c][:, 128 * a : 128 * (a + 1)],
                            S["x2T"][c][:],
                            start=(c == 0),
                            stop=(c == HT - 1),
                        )
                        if c == 0:
                            k = gate("psA", 3, inst)
                    nc.vector.tensor_copy(e12c[a][:], pe[:])
                    S["e12c_touch"].append(touch(e12c[a]))
                    nc.tensor.matmul(pe[:], onesb[:1, :], m2row, start=False,
                                     stop=True, skip_group_check=True)
                    stats(pe, p12[a], rz1[a])
                    set_gate("psA", k, gates["last_probs_touch"])
                return emit

            return [group(a) for a in range(NT)]

        def s4_e21(S, b):
            """e21 = transpose(e12 clean) + m1 rank-1, orientation-2 stats."""
            m1row = m1b[:, L * b : L * (b + 1)]
            p21 = [pp.tile([128, L], F16, name="p21", tag="p21")
                   for _ in range(NT)]
            rz2 = [st.tile([128, 1], F32, name="rz2", tag="rz2")
                   for _ in range(NT)]
            for c in range(NT):
                pe2 = psA.tile([128, L], F32, name="psA", tag="psA")
                inst = nc.tensor.matmul(pe2[:], onesb[:1, :], m1row,
                                        start=True, stop=True)
                k = gate("psA", 3, inst)
                add_dep_helper(inst.ins, S["e12c_touch"][-1].ins,
                               sync=False, reason="e12c touch gate")
                for a in range(NT):
                    nc.tensor.matmul(
                        pe2[:, 128 * a : 128 * (a + 1)],
                        S["e12c"][a][:, 128 * c : 128 * (c + 1)],
                        ident[:],
                        is_transpose=True,
                        start=False,
                        stop=True,
                        skip_group_check=True,
                    )
                stats(pe2, p21[c], rz2[c])
                set_gate("psA", k, gates["last_probs_touch"])
            S["p21"], S["rz2"] = p21, rz2

        def s5_pt(S, which):
            """bf16 PE transpose of probs -> contraction dim on partitions."""
            srcp = S[which]
            dstT = [ptp.tile([128, L], F16, name=which + "T", tag=which + "T")
                    for _ in range(NT)]
            for c in range(NT):
                tt = psTP.tile([128, L], F32, name="psTP", tag="psTP")
                ttb = tt[:].bitcast(F16)[:, 0:L]
                k = None
                for a in range(NT):
                    inst = nc.tensor.matmul(
                        ttb[:, 128 * a : 128 * (a + 1)],
                        srcp[a][:, 128 * c : 128 * (c + 1)],
                        identb[:],
                        is_transpose=True,
                    )
                    if a == 0:
                        k = gate("psTP", 2, inst)
                nc.scalar.copy(dstT[c][:], ttb[:])
                set_gate("psTP", k, touch(dstT[c]))
            S[which + "T"] = dstT

        def s6_stage2(S, b, which):
            """tilde = probsT^T @ x, normalize, enhance, DMA out (bf16).

            Returns 4 thunks (one per 128-row output tile)."""
            if which == "p12":
                pT, vals, xnat, rzs, y = S["p12T"], S["xn2"], S["xn1"], S["rz1"], y1
            else:
                pT, vals, xnat, rzs, y = S["p21T"], S["xn1"], S["xn2"], S["rz2"], y2

            yv = y.rearrange("b (q two p) c -> b q p two c", two=2, p=128)

            def group(q):
                def emit():
                    # two 128-row tiles per SBUF tile -> one merged DMA
                    ys2 = yp.tile([128, 2 * 4 * H], F16, name="ys", tag="ys")
                    nc.vector.memset(ys2[0:1, 0:1], 0.0)
                    for half in range(2):
                        a = 2 * q + half
                        ys = ys2[:, 4 * H * half : 4 * H * (half + 1)]
                        nc.gpsimd.tensor_copy(ys[:, 0:H], xnat[a][:])
                        for n in range(2):
                            pt = psB.tile([128, 512], F32, name="psB", tag="psB")
                            k = None
                            for c in range(NT):
                                inst = nc.tensor.matmul(
                                    pt[:],
                                    pT[c][:, 128 * a : 128 * (a + 1)],
                                    vals[c][:, 512 * n : 512 * (n + 1)],
                                    start=(c == 0),
                                    stop=(c == NT - 1),
                                )
                                if c == 0:
                                    k = gate("psB", 2, inst)
                            cols = slice(H + 512 * n, H + 512 * (n + 1))
                            if n == 0:
                                nc.scalar.activation(ys[:, cols], pt[:], Copy,
                                                     scale=rzs[a][:])
                            else:
                                nc.vector.tensor_scalar_mul(ys[:, cols], pt[:],
                                                            rzs[a][:])
                            set_gate("psB", k, touch(ys[:, cols]))
                        nc.vector.tensor_sub(ys[:, 2 * H : 3 * H], ys[:, 0:H],
                                             ys[:, H : 2 * H])
                        nc.gpsimd.tensor_mul(ys[:, 3 * H : 4 * H], ys[:, 0:H],
                                             ys[:, H : 2 * H])
                    nc.sync.dma_start(
                        yv[b, q],
                        ys2[:].rearrange("p (two c) -> p two c", two=2))
                return emit

            return [group(q) for q in range(NT // 2)]

        # -------- software-pipelined batch loop --------
        # PE order per iteration (b >= 1):
        #   pT transposes or1(b-1); then the 16 x-transpose groups of b
        #   interleaved with the 4 or1(b-1) output tiles; then pT or2(b-1);
        #   then the 4 e12 groups of b interleaved with the 4 or2(b-1)
        #   output tiles; then e21(b).
        # Outputs materialize evenly through the iteration, keeping the DMA
        # pipe fed, while the logits of b cover the softmax-stats latency of
        # b.  Input DMAs for b are queued at iteration start.
        def interleave(work, outs):
            """Merge work/outs thunk lists, spreading outs evenly."""
            if not outs:
                for w in work:
                    w()
                return
            stride = max(1, len(work) // len(outs))
            wi = 0
            for o in outs:
                for w in work[wi : wi + stride]:
                    w()
                wi += stride
                o()
            for w in work[wi:]:
                w()

        # Iteration k (steady state):
        #   s5b(k-1); {out2(k-1) x s2(k)}; loads(k+1); s3(k); s4(k);
        #   s5a(k); out1(k)
        # Orientation-2 outputs of k-1 stream while the x-transposes of k
        # run; the input DMAs for k+1 transfer during the (output-free)
        # logits phase; orientation-1 outputs of k stream at iteration end.
        states = {}
        held = []  # deferred output thunks from the previous iteration
        for b in range(BPC):
            if b == 0:
                S = states[0] = s1_load(0, halves=True)
                # warm the PE's view of the mask rows before their first use
                nc.tensor.matmul(scratch[0:32, 0:1],
                                 m1b[0:1, 0:32], onesb[0:1, 0:1],
                                 start=True, stop=True)
                nc.tensor.matmul(scratch[0:32, 0:1],
                                 m2b[0:1, 0:32], onesb[0:1, 0:1],
                                 start=True, stop=True)
                interleave(s2_xt(S), [])
            else:
                S = states[b]
                if b + 1 < BPC:
                    states[b + 1] = s1_load(b + 1)
                PS = states[b - 1]
                s5_pt(PS, "p21")
                out2 = s6_stage2(PS, b - 1, "p21")
                interleave(s2_xt(S), out2)
            if b == 0:
                states[1] = s1_load(1)
            # stream a deferred output from b-1 through the (otherwise
            # output-free) logits phase
            interleave(s3_e12(S, b), held)
            held = []
            if b == 0:
                # batch 0: get the first outputs moving before e21
                s5_pt(S, "p12")
                out1 = s6_stage2(S, b, "p12")
                out1[0]()
                s4_e21(S, b)
                held = out1[1:]
            elif b < BPC - 1:
                s4_e21(S, b)
                s5_pt(S, "p12")
                out1 = s6_stage2(S, b, "p12")
                out1[0]()
                held = out1[1:]
            else:
                # last batch: start the orientation-1 outputs right after
                # their stats so the closing DMA drain overlaps e21 and the
                # final stage-2 work
                s5_pt(S, "p12")
                out1 = s6_stage2(S, b, "p12")
                out1[0]()
                s4_e21(S, b)
                out1[1]()
                s5_pt(S, "p21")
                out2 = s6_stage2(S, b, "p21")
                for o in out2:
                    o()
    if not nc.is_finalized():
        nc.finalize()
    return nc


NEGH = np.float16(-6.0e4)  # effective -inf for fp16 mask rows


def make_core_inputs(x1, l1, x2, l2):
    """Build the in_map for one core's shard (BPC batches)."""
    ar = np.arange(L, dtype=np.int32)
    m1 = np.where(ar[None, :] >= np.asarray(l1)[:, None], NEGH, np.float16(0))
    m2 = np.where(ar[None, :] >= np.asarray(l2)[:, None], NEGH, np.float16(0))
    return {"x1": np.asarray(x1, np.float16),
            "x2": np.asarray(x2, np.float16),
            "m1": m1.astype(np.float16), "m2": m2.astype(np.float16)}


def kernel(x1_bar, seq_lengths1, x2_bar, seq_lengths2):
    x1_bar = np.ascontiguousarray(x1_bar, dtype=np.float32)
    x2_bar = np.ascontiguousarray(x2_bar, dtype=np.float32)

    if "nc" not in _NC_CACHE:
        _NC_CACHE["nc"] = build_nc()
    nc = _NC_CACHE["nc"]

    in_maps = []
    for c in range(NCORES):
        s = slice(c * BPC, (c + 1) * BPC)
        in_maps.append(make_core_inputs(
            x1_bar[s], np.asarray(seq_lengths1)[s],
            x2_bar[s], np.asarray(seq_lengths2)[s]))

    res = run_bass_kernel_spmd(nc, in_maps, core_ids=list(range(NCORES)))
    y1 = np.concatenate([np.asarray(r["y1"], np.float32) for r in res.results],
                        axis=0)
    y2 = np.concatenate([np.asarray(r["y2"], np.float32) for r in res.results],
                        axis=0)
    return y1, y2
